# revision 1
# baseline (speedup 1.0000x reference)
"""Builder for the DeepConvLSTM Trainium2 kernel (per-core program).

Per-core shapes: x [64,128,1,64] fp32 -> y [64,6] fp32.
Layouts:
  X0..X3 feature maps: [Cpart, (cblk,) B=64, TP=132] fp16, t padded by 2 each side.
  X4 chunk:            [128, 4 cblk, 4 b, 128 t] fp16 (per 4-sample chunk).
  xp1/xp2:             [128 gpart, 4 gate(i,f,o,g), 128 t, 64 b] fp16 (bias folded in).
  hr1 (relu lstm1 out):[128 h, 128 t, 64 b] fp16.
LSTM state: ST = [128, 128] fp16 = [tanh(g) | c]; H = [128 h, 64 b] fp16.
Gate source order in weights is Keras (i,f,g,o); we emit target order (i,f,o,g)
so sigmoid covers one contiguous [0:192] range and tanh(g) covers [192:256].
"""
import sys

sys.path.insert(0, "/opt/trn_rl_repo")
from contextlib import ExitStack

import concourse.bass as bass
import concourse.tile as tile
from concourse import bacc, mybir
from concourse.bass import ds, ts
from concourse.masks import make_identity

F32 = mybir.dt.float32
F16 = mybir.dt.float16
AF = mybir.ActivationFunctionType
OP = mybir.AluOpType

B = 64          # samples per core
T = 128         # time steps
TP = T + 4      # padded
H = 128         # lstm hidden
SRC = [0, 1, 3, 2]  # target gate j (i,f,o,g) -> source gate col block (i,f,g,o)


def build_program(n_cores=8, debug=False):
    nc = bacc.Bacc("TRN2", target_bir_lowering=False, debug=False,
                   num_devices=n_cores)
    ap = {}
    ap["x"] = nc.dram_tensor("x", [B, T, 1, 64], F32, kind="ExternalInput").ap()
    for name, shape in [
        ("conv1_w", [5, 5, 64, 64]), ("conv2_w", [5, 5, 64, 128]),
        ("conv3_w", [5, 5, 128, 256]), ("conv4_w", [5, 5, 256, 512]),
        ("lstm1_wx", [512, 512]), ("lstm1_wh", [128, 512]),
        ("lstm2_wx", [128, 512]), ("lstm2_wh", [128, 512]),
        ("dense_w", [128, 6]),
    ]:
        ap[name] = nc.dram_tensor(name, shape, F32, kind="ExternalInput").ap()
    for name, n in [("conv1_b", 64), ("conv2_b", 128), ("conv3_b", 256),
                    ("conv4_b", 512), ("lstm1_b", 512), ("lstm2_b", 512),
                    ("dense_b", 6)]:
        ap[name] = nc.dram_tensor(name, [n], F32, kind="ExternalInput").ap()
    y_d = nc.dram_tensor("y", [B, 6], F32, kind="ExternalOutput").ap()

    dbg = {}
    if debug:
        for name, shape in [("dbg_X0", [64, B, TP]), ("dbg_X1", [64, B, TP]),
                            ("dbg_X2", [128, B, TP]), ("dbg_X3", [128, 2, B, TP]),
                            ("dbg_xp1", [128, 4, T, B]), ("dbg_hr1", [128, T, B]),
                            ("dbg_xp2", [128, 4, T, B])]:
            dbg[name] = nc.dram_tensor(name, shape, F16, kind="ExternalOutput").ap()

    with tile.TileContext(nc) as tc, ExitStack() as ctx:
        _body(ctx, tc, ap, y_d, dbg)
    nc.compile()
    return nc


def _body(ctx, tc, ap, y_d, dbg):
    nc = tc.nc

    # ---------------- pools ----------------
    wpool = ctx.enter_context(tc.tile_pool(name="weights", bufs=1))
    featX3 = ctx.enter_context(tc.tile_pool(name="featX3", bufs=1))
    x4pool = ctx.enter_context(tc.tile_pool(name="x4c", bufs=2))
    hrpool = ctx.enter_context(tc.tile_pool(name="hr", bufs=1))
    small = ctx.enter_context(tc.tile_pool(name="small", bufs=4))
    state = ctx.enter_context(tc.tile_pool(name="state", bufs=1))
    cpsum = ctx.enter_context(tc.tile_pool(name="cpsum", bufs=4, space="PSUM"))
    featS_ctx = ExitStack()
    featS = featS_ctx.enter_context(tc.tile_pool(name="featS", bufs=2))
    stag_ctx = ExitStack()
    stag = stag_ctx.enter_context(tc.tile_pool(name="stag", bufs=2))
    xcpool = stag_ctx.enter_context(tc.tile_pool(name="xc16", bufs=1))

    # ---------------- weights: DMA + cast to fp16 ----------------
    ident = wpool.tile([128, 128], F16, tag="ident")
    make_identity(nc, ident[:])

    def stage_cast(dst_ap, src_ap, shape):
        st = stag.tile(list(shape), F32, tag="stag")
        nc.sync.dma_start(st[:], src_ap)
        nc.gpsimd.tensor_copy(dst_ap, st[:])

    wt1 = wpool.tile([64, 5, 64], F16, tag="wt1")
    stage_cast(wt1[:], ap["conv1_w"][:, 2, :, :].rearrange("k p co -> p k co"),
               [64, 5, 64])
    wt2 = wpool.tile([64, 5, 128], F16, tag="wt2")
    stage_cast(wt2[:], ap["conv2_w"][:, 2, :, :].rearrange("k p co -> p k co"),
               [64, 5, 128])
    wt3 = wpool.tile([128, 5, 256], F16, tag="wt3")
    stage_cast(wt3[:], ap["conv3_w"][:, 2, :, :].rearrange("k p co -> p k co"),
               [128, 5, 256])
    wt4 = wpool.tile([128, 5, 2, 512], F16, tag="wt4")
    for k in range(5):
        stage_cast(wt4[:, k], ap["conv4_w"][k, 2].rearrange("(cb p) co -> p cb co", p=128),
                   [128, 2, 512])
    wx1t = wpool.tile([128, 4, 512], F16, tag="wx1t")
    for db in range(4):
        stage_cast(wx1t[:, db], ap["lstm1_wx"][ds(db * 128, 128), :], [128, 512])
    wh1t = wpool.tile([128, 512], F16, tag="wh1t")
    stage_cast(wh1t[:], ap["lstm1_wh"][:], [128, 512])
    wx2t = wpool.tile([128, 512], F16, tag="wx2t")
    stage_cast(wx2t[:], ap["lstm2_wx"][:], [128, 512])
    wh2t = wpool.tile([128, 512], F16, tag="wh2t")
    stage_cast(wh2t[:], ap["lstm2_wh"][:], [128, 512])
    wdt = wpool.tile([128, 6], F16, tag="wdt")
    stage_cast(wdt[:], ap["dense_w"][:], [128, 6])

    # biases (fp32, straight DMA)
    bc1 = wpool.tile([64, 1], F32, tag="bc1")
    nc.sync.dma_start(bc1[:], ap["conv1_b"].rearrange("(c p) -> p c", c=1))
    bc2 = wpool.tile([128, 1], F32, tag="bc2")
    nc.sync.dma_start(bc2[:], ap["conv2_b"].rearrange("(c p) -> p c", c=1))
    bc3 = wpool.tile([128, 2], F32, tag="bc3")
    nc.sync.dma_start(bc3[:], ap["conv3_b"].rearrange("(cb p) -> p cb", p=128))
    bc4 = wpool.tile([128, 4], F32, tag="bc4")
    nc.sync.dma_start(bc4[:], ap["conv4_b"].rearrange("(cb p) -> p cb", p=128))
    bl1 = wpool.tile([128, 4], F32, tag="bl1")
    nc.sync.dma_start(bl1[:], ap["lstm1_b"].rearrange("(g p) -> p g", p=128))
    bl2 = wpool.tile([128, 4], F32, tag="bl2")
    nc.sync.dma_start(bl2[:], ap["lstm2_b"].rearrange("(g p) -> p g", p=128))
    bd1 = wpool.tile([1, 6], F32, tag="bd1")
    nc.sync.dma_start(bd1[:], ap["dense_b"].rearrange("(p c) -> p c", p=1))
    bdt = wpool.tile([64, 6], F32, tag="bdt")
    nc.gpsimd.partition_broadcast(bdt[:], bd1[:])

    # ---------------- input load / transpose ----------------
    # x [B,T,1,64] -> X0 [64c, B, TP] fp16 (pad 2 each side of t)
    X0 = featS.tile([64, B, TP], F16, tag="featS")
    nc.vector.memset(X0[:, :, 0:2], 0.0)
    nc.vector.memset(X0[:, :, TP - 2:TP], 0.0)
    xc16 = xcpool.tile([128, B, 64], F16, tag="xc16")
    for q in range(4):
        st = stag.tile([128, 16, 64], F32, tag="stagx")
        src = ap["x"][ds(q * 16, 16), :, 0, :].rearrange("b t c -> t b c")
        nc.sync.dma_start(st[:], src)
        nc.gpsimd.tensor_copy(xc16[:, ds(q * 16, 16), :], st[:])
    with tc.tile_pool(name="tpsum", bufs=2, space="PSUM") as tpsum:
        for bg in range(16):
            tp = tpsum.tile([64, 512], F16, tag="tpsum")
            for j in range(4):
                nc.tensor.transpose(tp[:, ds(j * 128, 128)],
                                    xc16[:, bg * 4 + j, :], ident[:])
            nc.vector.tensor_copy(X0[:, ts(bg, 4), 2:TP - 2], tp[:])
    stag_ctx.close()

    # ---------------- conv helper ----------------
    def conv_fwd(Xin, cin_blks, cin_p, wt, bias, co_blks, Xout, act_engine):
        # Xin: [cin_p, (cin_blks,) B, TP]; wt: [cin_p, 5, (cin_blks,) co_tot]
        # Xout: [co_p, (co_blks,) B, TP]
        for cob in range(co_blks):
            for nb in range(16):
                co_p = Xout.shape[0]
                ps_full = cpsum.tile([128, 512], F32, tag="cpsum", name="ps_full")
                ps = ps_full[:co_p]
                n_acc = 5 * cin_blks
                i = 0
                for k in range(5):
                    for cb in range(cin_blks):
                        if cin_blks == 1 and co_blks == 1:
                            lhsT = wt[:, k, :]
                            rhs = Xin[:, ts(nb, 4), ds(k, T)]
                        elif cin_blks == 1:
                            lhsT = wt[:, k, ds(cob * 128, 128)]
                            rhs = Xin[:, ts(nb, 4), ds(k, T)]
                        else:
                            lhsT = wt[:, k, cb, ds(cob * 128, 128)]
                            rhs = Xin[:, cb, ts(nb, 4), ds(k, T)]
                        nc.tensor.matmul(ps[:], lhsT, rhs,
                                         start=(i == 0), stop=(i == n_acc - 1))
                        i += 1
                if co_blks == 1:
                    out = Xout[:, ts(nb, 4), 2:TP - 2]
                else:
                    out = Xout[:, cob, ts(nb, 4), 2:TP - 2]
                if act_engine == "act":
                    nc.scalar.activation(out, ps[:], AF.Relu, bias=bias[:co_p, cob:cob + 1])
                else:
                    nc.vector.tensor_scalar(out, ps[:], bias[:co_p, cob:cob + 1], 0.0,
                                            op0=OP.add, op1=OP.max)

    X1 = featS.tile([64, B, TP], F16, tag="featS")
    nc.vector.memset(X1[:, :, 0:2], 0.0)
    nc.vector.memset(X1[:, :, TP - 2:TP], 0.0)
    conv_fwd(X0, 1, 64, wt1, bc1, 1, X1, "vector")

    X2 = featS.tile([128, B, TP], F16, tag="featS")
    nc.vector.memset(X2[:, :, 0:2], 0.0)
    nc.vector.memset(X2[:, :, TP - 2:TP], 0.0)
    conv_fwd(X1, 1, 64, wt2, bc2, 1, X2, "act")

    X3 = featX3.tile([128, 2, B, TP], F16, tag="featX3")
    nc.vector.memset(X3[:, :, :, 0:2], 0.0)
    nc.vector.memset(X3[:, :, :, TP - 2:TP], 0.0)
    conv_fwd(X2, 1, 128, wt3, bc3, 2, X3, "vector")

    featS_ctx.close()

    # ---------------- conv4 + xp1, chunked over b ----------------
    xppool = ctx.enter_context(tc.tile_pool(name="xp", bufs=1))
    xp1 = xppool.tile([128, 4, T, B], F16, tag="xp")
    for nb in range(16):
        X4c = x4pool.tile([128, 4, 4, T], F16, tag="x4c")
        for cob in range(4):
            ps = cpsum.tile([128, 512], F32, tag="cpsum")
            i = 0
            for k in range(5):
                for cb in range(2):
                    nc.tensor.matmul(ps[:], wt4[:, k, cb, ds(cob * 128, 128)],
                                     X3[:, cb, ts(nb, 4), ds(k, T)],
                                     start=(i == 0), stop=(i == 9))
                    i += 1
            eng = "act" if cob % 2 == 0 else "vec"
            if eng == "act":
                nc.scalar.activation(X4c[:, cob], ps[:], AF.Relu, bias=bc4[:, cob:cob + 1])
            else:
                nc.vector.tensor_scalar(X4c[:, cob], ps[:], bc4[:, cob:cob + 1], 0.0,
                                        op0=OP.add, op1=OP.max)
        for gb in range(4):
            ps = cpsum.tile([128, 512], F32, tag="cpsum")
            for db in range(4):
                nc.tensor.matmul(ps[:], wx1t[:, db, ds(SRC[gb] * 128, 128)],
                                 X4c[:, db], start=(db == 0), stop=(db == 3))
            out = xp1[:, gb, :, ts(nb, 4)].rearrange("p t b -> p b t")
            nc.scalar.activation(out, ps[:], AF.Identity, bias=bl1[:, SRC[gb]:SRC[gb] + 1])

    # ---------------- lstm step ----------------
    zpsum = ctx.enter_context(tc.tile_pool(name="zpsum", bufs=3, space="PSUM"))

    def lstm_step(xp, t, wht, ST, H_prev, hr_out):
        z = zpsum.tile([128, 256], F32, tag="z")
        nc.tensor.matmul(z[:], ident[:], xp[:, :, t, :], start=True, stop=False)
        for j in range(4):
            nc.tensor.matmul(z[:, ds(j * 64, 64)], wht[:, ds(SRC[j] * 128, 128)],
                             H_prev[:], start=False, stop=(j == 3))
        S = small.tile([128, 192], F16, tag="S")
        nc.scalar.activation(S[:], z[:, 0:192], AF.Sigmoid)
        nc.scalar.activation(ST[:, 0:64], z[:, 192:256], AF.Tanh)
        Pt = small.tile([128, 128], F16, tag="Pt")
        nc.vector.tensor_mul(Pt[:], S[:, 0:128], ST[:])
        nc.vector.tensor_add(ST[:, 64:128], Pt[:, 0:64], Pt[:, 64:128])
        TC = small.tile([128, 64], F16, tag="TC")
        nc.scalar.activation(TC[:], ST[:, 64:128], AF.Tanh)
        Hn = small.tile([128, 64], F16, tag="H")
        nc.vector.tensor_mul(Hn[:], S[:, 128:192], TC[:])
        if hr_out is not None:
            nc.vector.tensor_scalar(hr_out, Hn[:], 0.0, None, op0=OP.max)
        return Hn

    # ---- LSTM1 ----
    hr1 = hrpool.tile([128, T, B], F16, tag="hr")
    ST1 = state.tile([128, 128], F16, tag="ST1")
    nc.vector.memset(ST1[:, 64:128], 0.0)
    H1 = small.tile([128, 64], F16, tag="H")
    nc.vector.memset(H1[:], 0.0)
    for t in range(T):
        H1 = lstm_step(xp1, t, wh1t, ST1, H1, hr1[:, t, :])

    # ---- xp2 bulk: xp2 = hr1 @ wx2 + b2 ----
    xp2 = xppool.tile([128, 4, T, B], F16, tag="xp")
    for gb in range(4):
        for nb in range(16):
            ps = cpsum.tile([128, 512], F32, tag="cpsum")
            nc.tensor.matmul(ps[:], wx2t[:, ds(SRC[gb] * 128, 128)],
                             hr1[:, :, ts(nb, 4)], start=True, stop=True)
            out = xp2[:, gb, :, ts(nb, 4)]
            nc.vector.tensor_scalar(out, ps[:], bl2[:, SRC[gb]:SRC[gb] + 1], None, op0=OP.add)

    # ---- LSTM2 ----
    ST2 = state.tile([128, 128], F16, tag="ST2")
    nc.vector.memset(ST2[:, 64:128], 0.0)
    H2 = small.tile([128, 64], F16, tag="H")
    nc.vector.memset(H2[:], 0.0)
    for t in range(T):
        H2 = lstm_step(xp2, t, wh2t, ST2, H2, None)

    # ---- dense head ----
    rh2 = small.tile([128, 64], F16, tag="H")
    nc.vector.tensor_scalar(rh2[:], H2[:], 0.0, None, op0=OP.max)
    pd = zpsum.tile([128, 256], F32, tag="z")
    nc.tensor.matmul(pd[:64, 0:6], rh2[:], wdt[:], start=True, stop=True)
    yb = small.tile([64, 6], F32, tag="yb")
    nc.vector.tensor_add(yb[:], pd[:64, 0:6], bdt[:])
    ys = small.tile([64, 6], F32, tag="ys")
    nc.scalar.activation(ys[:], yb[:], AF.Sigmoid)
    nc.sync.dma_start(y_d[:], ys[:])

    # ---- debug outputs ----
    if dbg:
        nc.sync.dma_start(dbg["dbg_X0"][:], X0[:])
        nc.sync.dma_start(dbg["dbg_X1"][:], X1[:])
        nc.sync.dma_start(dbg["dbg_X2"][:], X2[:])
        nc.sync.dma_start(dbg["dbg_X3"][:], X3[:])
        nc.sync.dma_start(dbg["dbg_xp1"][:], xp1[:])
        nc.sync.dma_start(dbg["dbg_hr1"][:], hr1[:])
        nc.sync.dma_start(dbg["dbg_xp2"][:], xp2[:])


# ======================================================================
# Full-input kernel entry point: shard batch across 8 cores, run, gather.
# ======================================================================
import numpy as np

N_CORES = 8
_prog_cache = {}


def _get_program():
    if "nc" not in _prog_cache:
        _prog_cache["nc"] = build_program(n_cores=N_CORES, debug=False)
    return _prog_cache["nc"]


def kernel(**inputs):
    from concourse.bass_utils import run_bass_kernel_spmd

    nc = _get_program()
    x = np.ascontiguousarray(np.asarray(inputs["x"], dtype=np.float32))
    weights = {k: np.ascontiguousarray(np.asarray(v, dtype=np.float32))
               for k, v in inputs.items() if k != "x"}
    n = x.shape[0]
    per = n // N_CORES
    in_maps = []
    for c in range(N_CORES):
        m = {"x": x[c * per:(c + 1) * per]}
        m.update(weights)
        in_maps.append(m)
    res = run_bass_kernel_spmd(nc, in_maps, list(range(N_CORES)))
    out = np.concatenate([res.results[c]["y"] for c in range(N_CORES)], axis=0)
    return out.astype(np.float32)
